# revision 8
# baseline (speedup 1.0000x reference)
"""Trainium2 Bass kernel for nn_Critic (6-layer conv critic with line rasterization
and training-mode BatchNorm), data-parallel over 8 NeuronCores.

Strategy:
  - Host: rasterize the two endpoint pixels per sample into the line map
    (bit-exact replication of the jax reference, including its index -1 ->
    (255,255) wraparound), then build a 32-row im2col layout for conv0
    (K = 2ch x 16 taps) so the K=2 first conv becomes a K=32 matmul.
  - Device (per core, 8 samples): 5 phases.
      A: conv0 (4-sample tile_position row/col packing) + lrelu fused into
         PSUM eviction -> padded conv1 input; conv1 (2-sample row packing);
         per-channel sum/sumsq accumulated during eviction; h1 spilled to HBM.
      AllReduce #1 of BN1 stats (8 cores).
      B: load h1, apply BN1 affine + lrelu, pad; conv2; h2 stays in SBUF.
      AllReduce #2.  C: BN2+lrelu in place; conv3 (weights streamed from HBM).
      AllReduce #3.  D: BN3+lrelu; conv4 (weights streamed).
      AllReduce #4.  E: BN4+lrelu; conv5 (C_out=1).
  - Convs are computed as 16 shifted strided-AP matmuls accumulating in PSUM
    (channels on partitions); stride-2 selection is folded into the rhs AP.
"""

import os
import numpy as np

import concourse.bacc as bacc
import concourse.mybir as mybir
from concourse.tile import TileContext
from concourse.bass_utils import run_bass_kernel_spmd

F32 = mybir.dt.float32
MULT = mybir.AluOpType.mult
ADD = mybir.AluOpType.add
MAX = mybir.AluOpType.max
ACTF = mybir.ActivationFunctionType

N_CORES = 8
B = 64
H = W = 256
EPS = 1e-3

# knobs for the test harness (module-level; harness defaults are fine)
TRACE = False
TRACE_TMPDIR = None
LAST_EXEC_NS = None
SPC = 8            # samples per core
SIM = False        # build-only / simulated run (set by sim harness)

_TAPS = [(ky, kx) for ky in range(4) for kx in range(4)]


# --------------------------------------------------------------------------
# Host-side rasterization (bit-exact vs the jax reference on f32 math)
# --------------------------------------------------------------------------
def _rasterize_host(ch):
    flat = ch.reshape(-1)
    idx = np.nonzero(flat == 2.0)[0]
    i0, i1 = int(idx[0]), int(idx[1])
    y1, x1 = i0 // W, i0 % W
    y2, x2 = i1 // W, i1 % W
    ar = np.arange(256)
    dx, dy = x2 - x1, y2 - y1
    mx = (ar >= x1) & (ar < x2) if x1 < x2 else (ar > x2) & (ar <= x1)
    m1 = np.float32(dy) / np.float32(1 if dx == 0 else dx)
    ys = np.round(m1 * (ar - x1).astype(np.float32) + np.float32(y1)).astype(np.int64)
    my = (ar >= y1) & (ar < y2) if y1 < y2 else (ar > y2) & (ar <= y1)
    m2 = np.float32(dx) / np.float32(1 if dy == 0 else dy)
    xs = np.round(m2 * (ar - y1).astype(np.float32) + np.float32(x1)).astype(np.int64)
    ends = np.zeros((256, 256), np.float32)
    ends[ys[mx], ar[mx]] = 1.0
    ends[ar[my], xs[my]] = 1.0
    # the reference scatters masked-out lanes at index -1, which wraps
    if (~mx).any() or (~my).any():
        ends[255, 255] = 1.0
    ends[y1, x1] = 2.0
    ends[y2, x2] = 2.0
    return ends


# --------------------------------------------------------------------------
# Device program
# --------------------------------------------------------------------------
def build_nc(spc=8, b0_nonzero=False):
    """Build the per-core Bass program for `spc` samples (must be mult of 2)."""
    assert spc % 2 == 0
    nc = bacc.Bacc("TRN2", target_bir_lowering=False, debug=False,
                   num_devices=N_CORES)
    npair = spc // 2

    x0_d = nc.dram_tensor("x0", [npair, 128, 8192], F32, kind="ExternalInput")
    w0_d = nc.dram_tensor("w0", [128, 64], F32, kind="ExternalInput")
    w1_d = nc.dram_tensor("w1", [128, 16, 128], F32, kind="ExternalInput")
    w2_d = nc.dram_tensor("w2", [128, 16, 256], F32, kind="ExternalInput")
    w3_d = nc.dram_tensor("w3", [4, 2, 128, 16, 128], F32, kind="ExternalInput")
    w4_d = nc.dram_tensor("w4", [4, 4, 128, 16, 128], F32, kind="ExternalInput")
    w5_d = nc.dram_tensor("w5", [128, 4, 16], F32, kind="ExternalInput")
    bn1_d = nc.dram_tensor("bn1", [128, 1, 2], F32, kind="ExternalInput")
    bn2_d = nc.dram_tensor("bn2", [128, 2, 2], F32, kind="ExternalInput")
    bn3_d = nc.dram_tensor("bn3", [128, 4, 2], F32, kind="ExternalInput")
    bn4_d = nc.dram_tensor("bn4", [128, 4, 2], F32, kind="ExternalInput")
    y_d = nc.dram_tensor("y", [1, spc, 256], F32, kind="ExternalOutput")
    h1_dram = nc.dram_tensor("h1tmp", [spc, 128, 4096], F32)

    # batch-stat denominators (full batch across all 8 cores)
    N1 = float(B * 64 * 64)
    N2 = float(B * 32 * 32)
    N3 = float(B * 16 * 16)
    N4 = float(B * 16 * 16)

    with TileContext(nc) as tc:
        import contextlib
        es = contextlib.ExitStack()
        with es:
            persist = es.enter_context(tc.tile_pool(name="persist", bufs=1))
            stats = es.enter_context(tc.tile_pool(name="stats", bufs=1))
            ccdram = es.enter_context(tc.tile_pool(name="ccdram", bufs=1, space="DRAM"))

            # ---- persistent small tensors ----
            w0_sb = persist.tile([128, 64], F32)
            nc.sync.dma_start(out=w0_sb[:], in_=w0_d[:])
            w1_sb = persist.tile([128, 16, 128], F32)
            nc.sync.dma_start(out=w1_sb[:], in_=w1_d[:])
            w5_sb = persist.tile([128, 4, 16], F32)
            nc.sync.dma_start(out=w5_sb[:], in_=w5_d[:])
            bn_sb = {}
            for li, (d, nb) in {1: (bn1_d, 1), 2: (bn2_d, 2), 3: (bn3_d, 4), 4: (bn4_d, 4)}.items():
                t = persist.tile([128, nb, 2], F32, tag=f"bn{li}", name=f"bn{li}")
                nc.sync.dma_start(out=t[:], in_=d[:])
                bn_sb[li] = t
            eps_sb = persist.tile([128, 1], F32)
            nc.vector.memset(eps_sb[:], EPS)

            # per-layer BN scale/shift results [128, nb]
            scsh = {li: (persist.tile([128, nb], F32, tag=f"sc{li}", name=f"sc{li}"),
                         persist.tile([128, nb], F32, tag=f"sh{li}", name=f"sh{li}"))
                    for li, nb in {1: 1, 2: 2, 3: 4, 4: 4}.items()}

            # stat partial tiles
            part = {
                1: (stats.tile([128, 8 * spc], F32, tag="p1s", name="p1s"),
                    stats.tile([128, 8 * spc], F32, tag="p1q", name="p1q")),
                2: (stats.tile([128, 2, 2 * spc], F32, tag="p2s", name="p2s"),
                    stats.tile([128, 2, 2 * spc], F32, tag="p2q", name="p2q")),
                3: (stats.tile([128, 4, spc], F32, tag="p3s", name="p3s"),
                    stats.tile([128, 4, spc], F32, tag="p3q", name="p3q")),
                4: (stats.tile([128, 4, spc], F32, tag="p4s", name="p4s"),
                    stats.tile([128, 4, spc], F32, tag="p4q", name="p4q")),
            }

            def bn_reduce_allreduce(li, nb, n_elem):
                """partials -> AllReduce -> sc/sh [128, nb]."""
                ps, pq = part[li]
                cc_in = stats.tile([128, 2 * nb], F32, tag=f"ccin{li}", name=f"ccin{li}")
                for b in range(nb):
                    pss = ps[:] if nb == 1 else ps[:, b, :]
                    pqq = pq[:] if nb == 1 else pq[:, b, :]
                    nc.vector.tensor_reduce(out=cc_in[:, b:b + 1], in_=pss,
                                            axis=mybir.AxisListType.X, op=ADD)
                    nc.vector.tensor_reduce(out=cc_in[:, nb + b:nb + b + 1], in_=pqq,
                                            axis=mybir.AxisListType.X, op=ADD)
                din = ccdram.tile([128, 2 * nb], F32, tag=f"ccdin{li}", name=f"ccdin{li}")
                dout = ccdram.tile([128, 2 * nb], F32, tag=f"ccdout{li}", name=f"ccdout{li}")
                nc.gpsimd.dma_start(out=din[:], in_=cc_in[:])
                nc.gpsimd.collective_compute(
                    "AllReduce", ADD,
                    replica_groups=[list(range(N_CORES))],
                    ins=[din.opt()], outs=[dout.opt()],
                )
                g_sb = stats.tile([128, 2 * nb], F32, tag=f"ccg{li}", name=f"ccg{li}")
                nc.gpsimd.dma_start(out=g_sb[:], in_=dout[:])
                sc_t, sh_t = scsh[li]
                mean = stats.tile([128, nb], F32, tag=f"mean{li}", name=f"mean{li}")
                negm2 = stats.tile([128, nb], F32, tag=f"negm2{li}", name=f"negm2{li}")
                var = stats.tile([128, nb], F32, tag=f"var{li}", name=f"var{li}")
                for b in range(nb):
                    nc.vector.tensor_scalar_mul(mean[:, b:b + 1], g_sb[:, b:b + 1], 1.0 / n_elem)
                    nc.vector.scalar_tensor_tensor(out=negm2[:, b:b + 1], in0=mean[:, b:b + 1],
                                                   scalar=-1.0, in1=mean[:, b:b + 1],
                                                   op0=MULT, op1=MULT)
                    nc.vector.scalar_tensor_tensor(out=var[:, b:b + 1], in0=g_sb[:, nb + b:nb + b + 1],
                                                   scalar=1.0 / n_elem, in1=negm2[:, b:b + 1],
                                                   op0=MULT, op1=ADD)
                    # sd = sqrt(var + eps); rstd = 1/sd
                    nc.scalar.activation(out=var[:, b:b + 1], in_=var[:, b:b + 1],
                                         func=ACTF.Sqrt, bias=eps_sb[:], scale=1.0)
                    nc.vector.reciprocal(out=var[:, b:b + 1], in_=var[:, b:b + 1])
                    gcol = bn_sb[li][:, b, 0:1]
                    becol = bn_sb[li][:, b, 1:2]
                    nc.vector.tensor_mul(sc_t[:, b:b + 1], var[:, b:b + 1], gcol)
                    # sh = be - mean * sc
                    nc.vector.tensor_mul(mean[:, b:b + 1], mean[:, b:b + 1], sc_t[:, b:b + 1])
                    nc.vector.scalar_tensor_tensor(out=sh_t[:, b:b + 1], in0=mean[:, b:b + 1],
                                                   scalar=-1.0, in1=becol,
                                                   op0=MULT, op1=ADD)

            # ============================ PHASE A ============================
            with tc.tile_pool(name="pa_x", bufs=2) as pa_x, \
                 tc.tile_pool(name="pa_p1", bufs=1) as pa_p1, \
                 tc.tile_pool(name="pa_h1", bufs=2) as pa_h1, \
                 tc.tile_pool(name="pa_scr", bufs=2) as pa_scr, \
                 tc.tile_pool(name="ps_l0", bufs=3, space="PSUM") as ps_l0, \
                 tc.tile_pool(name="ps_l1", bufs=4, space="PSUM") as ps_l1:
                s1p, s1q = part[1]
                for pr in range(npair):
                    xp = pa_x.tile([128, 8192], F32, tag="x0", name="xp")
                    for q in range(2):
                        nc.sync.dma_start(out=xp[:, 4096 * q:4096 * (q + 1)],
                                          in_=x0_d[pr, :, 4096 * q:4096 * (q + 1)])
                    p1t = pa_p1.tile([128, 16900], F32, tag="p1", name="p1t")
                    pv = p1t.rearrange("p (h w) -> p h w", h=130)
                    nc.gpsimd.memset(pv[:, 0, :], 0.0)
                    nc.gpsimd.memset(pv[:, 129, :], 0.0)
                    nc.gpsimd.memset(pv[:, 1:129, 0:130:129], 0.0)
                    # ---- conv0: 2 samples packed (rows = K, cols = out half) ----
                    for c in range(32):
                        half = c // 16
                        poff = 64 * half
                        cc = c % 16
                        psA = ps_l0.tile([128, 512], F32, tag="l0", name="psl0")
                        nc.tensor.matmul(psA[0:64, :],
                                         w0_sb[poff:poff + 32, :],
                                         xp[poff:poff + 32, 512 * cc:512 * (cc + 1)],
                                         start=True, stop=True,
                                         tile_position=(poff, 0))
                        nc.tensor.matmul(psA[64:128, :],
                                         w0_sb[poff + 32:poff + 64, :],
                                         xp[poff + 32:poff + 64, 512 * cc:512 * (cc + 1)],
                                         start=True, stop=True,
                                         tile_position=(poff + 32, 64))
                        dst = pv[:, 4 * c + 1:4 * c + 5, 1:129]
                        nc.scalar.activation(out=dst,
                                             in_=psA[:].rearrange("p (a b) -> p a b", a=4),
                                             func=ACTF.Copy)
                        nc.vector.scalar_tensor_tensor(out=dst, in0=dst, scalar=0.2,
                                                       in1=dst, op0=MULT, op1=MAX)
                    # ---- conv1: 2 samples packed on partition halves ----
                    h1s = [pa_h1.tile([128, 4096], F32, tag="h1", name="h1s")
                           for _ in range(2)]
                    for grp in range(4):
                        pss = [[ps_l1.tile([128, 512], F32, tag="l1", name="psl1")
                                for _ in range(2)] for _ in range(2)]
                        for tap, (ky, kx) in enumerate(_TAPS):
                            for s in range(2):
                                lhsT = w1_sb[64 * s:64 * s + 64, tap, :]
                                for ch in range(2):
                                    chunk = grp * 2 + ch
                                    rhs = pv[64 * s:64 * s + 64,
                                             ky + 16 * chunk:ky + 16 * chunk + 15:2,
                                             kx:kx + 127:2]
                                    nc.tensor.matmul(pss[s][ch][:], lhsT, rhs,
                                                     start=(tap == 0), stop=(tap == 15),
                                                     tile_position=(64 * s, 0))
                        for s in range(2):
                            sg = 2 * pr + s
                            for ch in range(2):
                                chunk = grp * 2 + ch
                                slot = sg * 8 + chunk
                                nc.scalar.activation(
                                    out=h1s[s][:, 512 * chunk:512 * (chunk + 1)],
                                    in_=pss[s][ch][:], func=ACTF.Copy,
                                    accum_out=s1p[:, slot:slot + 1])
                                scr = pa_scr.tile([128, 512], F32, tag="scr", name="scr")
                                nc.scalar.activation(
                                    out=scr[:], in_=pss[s][ch][:], func=ACTF.Square,
                                    accum_out=s1q[:, slot:slot + 1])
                    for s in range(2):
                        sg = 2 * pr + s
                        nc.sync.dma_start(out=h1_dram[sg], in_=h1s[s][:])

            bn_reduce_allreduce(1, 1, N1)

            # ============================ PHASE B ============================
            sc1, sh1 = scsh[1]
            ph2 = tc.alloc_tile_pool(name="ph2", bufs=1)
            h2 = [ph2.tile([128, spc, 1156], F32, tag=f"h2_{kb}", name=f"h2_{kb}")
                  for kb in range(2)]
            h2v = [h.rearrange("p s (h w) -> p s h w", h=34) for h in h2]
            for kb in range(2):
                nc.gpsimd.memset(h2v[kb][:, :, 0, :], 0.0)
                nc.gpsimd.memset(h2v[kb][:, :, 33, :], 0.0)
                nc.gpsimd.memset(h2v[kb][:, :, 1:33, 0], 0.0)
                nc.gpsimd.memset(h2v[kb][:, :, 1:33, 33], 0.0)
            with tc.tile_pool(name="pb_in", bufs=2) as pb_in, \
                 tc.tile_pool(name="pb_p2", bufs=2) as pb_p2, \
                 tc.tile_pool(name="pb_w", bufs=1) as pb_w, \
                 tc.tile_pool(name="pb_scr", bufs=2) as pb_scr, \
                 tc.tile_pool(name="ps_l2", bufs=8, space="PSUM") as ps_l2:
                w2_sb = pb_w.tile([128, 16, 256], F32)
                nc.sync.dma_start(out=w2_sb[:], in_=w2_d[:])
                s2p, s2q = part[2]
                for s in range(spc):
                    lb = pb_in.tile([128, 4096], F32, tag="lb", name="lb")
                    nc.sync.dma_start(out=lb[:], in_=h1_dram[s])
                    p2t = pb_p2.tile([128, 4356], F32, tag="p2", name="p2t")
                    p2 = p2t.rearrange("p (h w) -> p h w", h=66)
                    nc.gpsimd.memset(p2[:, 0, :], 0.0)
                    nc.gpsimd.memset(p2[:, 65, :], 0.0)
                    nc.gpsimd.memset(p2[:, 1:65, 0:66:65], 0.0)
                    interior = p2[:, 1:65, 1:65]
                    nc.vector.tensor_scalar(out=interior,
                                            in0=lb[:].rearrange("p (h w) -> p h w", h=64),
                                            scalar1=sc1[:, 0:1], scalar2=sh1[:, 0:1],
                                            op0=MULT, op1=ADD)
                    nc.vector.scalar_tensor_tensor(out=interior, in0=interior, scalar=0.2,
                                                   in1=interior, op0=MULT, op1=MAX)
                    for mt in range(2):
                        pss = [ps_l2.tile([128, 512], F32, tag="l2", name="psl2") for _ in range(2)]
                        for tap, (ky, kx) in enumerate(_TAPS):
                            lhsT = w2_sb[:, tap, 128 * mt:128 * mt + 128]
                            for ch in range(2):
                                rhs = p2[:, ky + 32 * ch:ky + 32 * ch + 31:2, kx:kx + 63:2]
                                nc.tensor.matmul(pss[ch][:], lhsT, rhs,
                                                 start=(tap == 0), stop=(tap == 15))
                        for ch in range(2):
                            slot = s * 2 + ch
                            dst = h2v[mt][:, s, 16 * ch + 1:16 * ch + 17, 1:33]
                            nc.scalar.activation(out=dst,
                                                 in_=pss[ch][:].rearrange("p (a b) -> p a b", a=16),
                                                 func=ACTF.Copy,
                                                 accum_out=s2p[:, mt, slot:slot + 1])
                            scr = pb_scr.tile([128, 512], F32, tag="scr", name="scr")
                            nc.scalar.activation(out=scr[:], in_=pss[ch][:], func=ACTF.Square,
                                                 accum_out=s2q[:, mt, slot:slot + 1])

            bn_reduce_allreduce(2, 2, N2)

            # ============================ PHASE C ============================
            sc2, sh2 = scsh[2]
            for kb in range(2):
                for s in range(spc):
                    inter = h2v[kb][:, s, 1:33, 1:33]
                    nc.vector.tensor_scalar(out=inter, in0=inter, scalar1=sc2[:, kb:kb + 1],
                                            scalar2=sh2[:, kb:kb + 1], op0=MULT, op1=ADD)
                    nc.vector.scalar_tensor_tensor(out=inter, in0=inter, scalar=0.2,
                                                   in1=inter, op0=MULT, op1=MAX)
            ph3 = tc.alloc_tile_pool(name="ph3", bufs=1, side="right")
            h3 = [ph3.tile([128, spc, 361], F32, tag=f"h3_{kb}", name=f"h3_{kb}")
                  for kb in range(4)]
            h3v = [h.rearrange("p s (h w) -> p s h w", h=19) for h in h3]
            for kb in range(4):
                nc.gpsimd.memset(h3v[kb][:, :, 0, :], 0.0)
                nc.gpsimd.memset(h3v[kb][:, :, 17, :], 0.0)
                nc.gpsimd.memset(h3v[kb][:, :, 18, :], 0.0)
                nc.gpsimd.memset(h3v[kb][:, :, 1:17, 0], 0.0)
                nc.gpsimd.memset(h3v[kb][:, :, 1:17, 17], 0.0)
                nc.gpsimd.memset(h3v[kb][:, :, 1:17, 18], 0.0)
            with tc.tile_pool(name="pc_w", bufs=4) as pc_w, \
                 tc.tile_pool(name="pc_scr", bufs=2) as pc_scr, \
                 tc.tile_pool(name="ps_l3", bufs=8, space="PSUM") as ps_l3:
                s3p, s3q = part[3]
                for mt in range(4):
                    w3t = [pc_w.tile([128, 16, 128], F32, tag="w3", name="w3t") for _ in range(2)]
                    for kb in range(2):
                        nc.sync.dma_start(out=w3t[kb][:], in_=w3_d[mt, kb])
                    pss = [ps_l3.tile([128, 256], F32, tag="l3", name="psl3")
                           for _ in range(spc)]
                    for kb in range(2):
                        for tap, (ky, kx) in enumerate(_TAPS):
                            lhsT = w3t[kb][:, tap, :]
                            for s in range(spc):
                                rhs = h2v[kb][:, s, ky:ky + 31:2, kx:kx + 31:2]
                                nc.tensor.matmul(pss[s][:], lhsT, rhs,
                                                 start=(kb == 0 and tap == 0),
                                                 stop=(kb == 1 and tap == 15))
                    for s in range(spc):
                        dst = h3v[mt][:, s, 1:17, 1:17]
                        nc.scalar.activation(out=dst,
                                             in_=pss[s][:].rearrange("p (a b) -> p a b", a=16),
                                             func=ACTF.Copy, accum_out=s3p[:, mt, s:s + 1])
                        scr = pc_scr.tile([128, 512], F32, tag="scr", name="scr")
                        nc.scalar.activation(out=scr[:, 0:256], in_=pss[s][:], func=ACTF.Square,
                                             accum_out=s3q[:, mt, s:s + 1])
            ph2.release()

            bn_reduce_allreduce(3, 4, N3)

            # ============================ PHASE D ============================
            sc3, sh3 = scsh[3]
            for kb in range(4):
                for s in range(spc):
                    inter = h3v[kb][:, s, 1:17, 1:17]
                    nc.vector.tensor_scalar(out=inter, in0=inter, scalar1=sc3[:, kb:kb + 1],
                                            scalar2=sh3[:, kb:kb + 1], op0=MULT, op1=ADD)
                    nc.vector.scalar_tensor_tensor(out=inter, in0=inter, scalar=0.2,
                                                   in1=inter, op0=MULT, op1=MAX)
            ph4 = tc.alloc_tile_pool(name="ph4", bufs=1)
            h4 = [ph4.tile([128, spc, 361], F32, tag=f"h4_{kb}", name=f"h4_{kb}")
                  for kb in range(4)]
            h4v = [h.rearrange("p s (h w) -> p s h w", h=19) for h in h4]
            for kb in range(4):
                nc.gpsimd.memset(h4v[kb][:, :, 0, :], 0.0)
                nc.gpsimd.memset(h4v[kb][:, :, 17, :], 0.0)
                nc.gpsimd.memset(h4v[kb][:, :, 18, :], 0.0)
                nc.gpsimd.memset(h4v[kb][:, :, 1:17, 0], 0.0)
                nc.gpsimd.memset(h4v[kb][:, :, 1:17, 17], 0.0)
                nc.gpsimd.memset(h4v[kb][:, :, 1:17, 18], 0.0)
            with tc.tile_pool(name="pd_w", bufs=6) as pd_w, \
                 tc.tile_pool(name="pd_scr", bufs=2) as pd_scr, \
                 tc.tile_pool(name="ps_l4", bufs=8, space="PSUM") as ps_l4:
                s4p, s4q = part[4]
                for mt in range(4):
                    w4t = [pd_w.tile([128, 16, 128], F32, tag="w4", name="w4t") for _ in range(4)]
                    for kb in range(4):
                        nc.sync.dma_start(out=w4t[kb][:], in_=w4_d[mt, kb])
                    pss = [ps_l4.tile([128, 256], F32, tag="l4", name="psl4")
                           for _ in range(spc)]
                    for kb in range(4):
                        for tap, (ky, kx) in enumerate(_TAPS):
                            lhsT = w4t[kb][:, tap, :]
                            for s in range(spc):
                                rhs = h3v[kb][:, s, ky:ky + 16, kx:kx + 16]
                                nc.tensor.matmul(pss[s][:], lhsT, rhs,
                                                 start=(kb == 0 and tap == 0),
                                                 stop=(kb == 3 and tap == 15))
                    for s in range(spc):
                        dst = h4v[mt][:, s, 1:17, 1:17]
                        nc.scalar.activation(out=dst,
                                             in_=pss[s][:].rearrange("p (a b) -> p a b", a=16),
                                             func=ACTF.Copy, accum_out=s4p[:, mt, s:s + 1])
                        scr = pd_scr.tile([128, 512], F32, tag="scr", name="scr")
                        nc.scalar.activation(out=scr[:, 0:256], in_=pss[s][:], func=ACTF.Square,
                                             accum_out=s4q[:, mt, s:s + 1])
            ph3.release()

            bn_reduce_allreduce(4, 4, N4)

            # ============================ PHASE E ============================
            sc4, sh4 = scsh[4]
            for kb in range(4):
                for s in range(spc):
                    inter = h4v[kb][:, s, 1:17, 1:17]
                    nc.vector.tensor_scalar(out=inter, in0=inter, scalar1=sc4[:, kb:kb + 1],
                                            scalar2=sh4[:, kb:kb + 1], op0=MULT, op1=ADD)
                    nc.vector.scalar_tensor_tensor(out=inter, in0=inter, scalar=0.2,
                                                   in1=inter, op0=MULT, op1=MAX)
            with tc.tile_pool(name="pe_out", bufs=1) as pe_out, \
                 tc.tile_pool(name="ps_l5", bufs=8, space="PSUM") as ps_l5:
                ys = pe_out.tile([1, spc, 256], F32)
                pse = [ps_l5.tile([1, 256], F32, tag="l5", name="psl5") for s in range(spc)]
                for kb in range(4):
                    for tap, (ky, kx) in enumerate(_TAPS):
                        lhsT = w5_sb[:, kb, tap:tap + 1]
                        for s in range(spc):
                            rhs = h4v[kb][:, s, ky:ky + 16, kx:kx + 16]
                            nc.tensor.matmul(pse[s][:], lhsT, rhs,
                                             start=(kb == 0 and tap == 0),
                                             stop=(kb == 3 and tap == 15))
                for s in range(spc):
                    nc.scalar.activation(out=ys[:, s, :], in_=pse[s][:], func=ACTF.Copy)
                nc.sync.dma_start(out=y_d[:], in_=ys[:])
            ph4.release()
    nc.compile()
    return nc


# --------------------------------------------------------------------------
# Host-side input prep
# --------------------------------------------------------------------------
def _prep_inputs(x, params, spc=8, cores=N_CORES):
    x = np.ascontiguousarray(np.asarray(x, np.float32))
    p = {k: np.asarray(v, np.float32) for k, v in params.items()}

    ends = np.stack([_rasterize_host(x[i, 1]) for i in range(x.shape[0])])
    xin = np.zeros((x.shape[0], 2, 258, 258), np.float32)
    xin[:, 0, 1:257, 1:257] = x[:, 0]
    xin[:, 1, 1:257, 1:257] = ends
    # im2col rows ordered (ky, kx, ci)
    cols = np.empty((x.shape[0], 4, 4, 2, 128, 128), np.float32)
    for ky in range(4):
        for kx in range(4):
            cols[:, ky, kx] = xin[:, :, ky:ky + 255:2, kx:kx + 255:2]
    imcol = cols.reshape(x.shape[0], 32, 16384)

    w0 = np.tile(p["w0"].transpose(2, 3, 1, 0).reshape(32, 64), (4, 1))
    w1 = np.tile(p["w1"].transpose(1, 2, 3, 0).reshape(64, 16, 128), (2, 1, 1))
    w2 = p["w2"].transpose(1, 2, 3, 0).reshape(128, 16, 256)
    w3 = (p["w3"].transpose(1, 2, 3, 0).reshape(2, 128, 16, 4, 128)
          .transpose(3, 0, 1, 2, 4))
    w4 = (p["w4"].transpose(1, 2, 3, 0).reshape(4, 128, 16, 4, 128)
          .transpose(3, 0, 1, 2, 4))
    w5 = p["w5"].transpose(1, 2, 3, 0).reshape(4, 128, 16).transpose(1, 0, 2)

    def bn_pack(li, nb):
        g = p[f"g{li}"].reshape(nb, 128)
        be = p[f"be{li}"].reshape(nb, 128)
        t = np.empty((128, nb, 2), np.float32)
        t[:, :, 0] = g.T
        t[:, :, 1] = be.T
        return t

    # NOTE: conv biases b1..b4 cancel inside training-mode BN (they shift the
    # mean by exactly b); b5 is added on the host after the gather; b0 is zero
    # in this problem's setup (asserted in kernel()).
    shared = {
        "w0": np.ascontiguousarray(w0), "w1": np.ascontiguousarray(w1),
        "w2": np.ascontiguousarray(w2), "w3": np.ascontiguousarray(w3),
        "w4": np.ascontiguousarray(w4), "w5": np.ascontiguousarray(w5),
        "bn1": bn_pack(1, 1), "bn2": bn_pack(2, 2),
        "bn3": bn_pack(3, 4), "bn4": bn_pack(4, 4),
    }
    in_maps = []
    for c in range(cores):
        sm = imcol[c * spc:(c + 1) * spc]  # [spc, 32, 16384]
        x0 = np.empty((spc // 2, 128, 8192), np.float32)
        for pr in range(spc // 2):
            x0[pr, 0:32] = sm[2 * pr, :, :8192]
            x0[pr, 32:64] = sm[2 * pr + 1, :, :8192]
            x0[pr, 64:96] = sm[2 * pr, :, 8192:]
            x0[pr, 96:128] = sm[2 * pr + 1, :, 8192:]
        m = dict(shared)
        m["x0"] = np.ascontiguousarray(x0)
        in_maps.append(m)
    return in_maps, p


_NC_CACHE = {}


def kernel(x, params):
    global LAST_EXEC_NS
    in_maps, p = _prep_inputs(x, params, spc=SPC)
    assert not np.any(p["b0"]), "conv0 bias path not implemented"
    key = (SPC,)
    if key not in _NC_CACHE:
        _NC_CACHE[key] = build_nc(spc=SPC)
    nc = _NC_CACHE[key]
    res = run_bass_kernel_spmd(
        nc, in_maps, list(range(N_CORES)),
        trace=TRACE, tmpdir=TRACE_TMPDIR,
    )
    LAST_EXEC_NS = res.exec_time_ns
    out = np.concatenate([res.results[c]["y"].reshape(SPC, 256) for c in range(N_CORES)], axis=0)
    out = out.reshape(B, 1, 16, 16) + p["b5"].reshape(1, 1, 1, 1)
    return out.astype(np.float32)


# revision 11
# speedup vs baseline: 2.1372x; 2.1372x over previous
"""Trainium2 Bass kernel for nn_Critic (6-layer conv critic with line rasterization
and training-mode BatchNorm), data-parallel over 8 NeuronCores.

Strategy:
  - Host: rasterize the two endpoint pixels per sample into the line map
    (bit-exact replication of the jax reference, including its index -1 ->
    (255,255) wraparound), then build a 32-row im2col layout for conv0
    (K = 2ch x 16 taps) so the K=2 first conv becomes a K=32 matmul.
  - Device (per core, 8 samples): 5 phases.
      A: conv0 (4-sample tile_position row/col packing) + lrelu fused into
         PSUM eviction -> padded conv1 input; conv1 (2-sample row packing);
         per-channel sum/sumsq accumulated during eviction; h1 spilled to HBM.
      AllReduce #1 of BN1 stats (8 cores).
      B: load h1, apply BN1 affine + lrelu, pad; conv2; h2 stays in SBUF.
      AllReduce #2.  C: BN2+lrelu in place; conv3 (weights streamed from HBM).
      AllReduce #3.  D: BN3+lrelu; conv4 (weights streamed).
      AllReduce #4.  E: BN4+lrelu; conv5 (C_out=1).
  - Convs are computed as 16 shifted strided-AP matmuls accumulating in PSUM
    (channels on partitions); stride-2 selection is folded into the rhs AP.
"""

import os
import numpy as np

import concourse.bacc as bacc
import concourse.mybir as mybir
from concourse.tile import TileContext
from concourse.bass_utils import run_bass_kernel_spmd

F32 = mybir.dt.float32
F32R = mybir.dt.float32r
MULT = mybir.AluOpType.mult
ADD = mybir.AluOpType.add
MAX = mybir.AluOpType.max
ACTF = mybir.ActivationFunctionType

N_CORES = 8
B = 64
H = W = 256
EPS = 1e-3

# knobs for the test harness (module-level; harness defaults are fine)
TRACE = False
TRACE_TMPDIR = None
LAST_EXEC_NS = None
SPC = 8            # samples per core
SIM = False        # build-only / simulated run (set by sim harness)

_TAPS = [(ky, kx) for ky in range(4) for kx in range(4)]


# --------------------------------------------------------------------------
# Host-side rasterization (bit-exact vs the jax reference on f32 math)
# --------------------------------------------------------------------------
def _rasterize_host(ch):
    flat = ch.reshape(-1)
    idx = np.nonzero(flat == 2.0)[0]
    i0, i1 = int(idx[0]), int(idx[1])
    y1, x1 = i0 // W, i0 % W
    y2, x2 = i1 // W, i1 % W
    ar = np.arange(256)
    dx, dy = x2 - x1, y2 - y1
    mx = (ar >= x1) & (ar < x2) if x1 < x2 else (ar > x2) & (ar <= x1)
    m1 = np.float32(dy) / np.float32(1 if dx == 0 else dx)
    ys = np.round(m1 * (ar - x1).astype(np.float32) + np.float32(y1)).astype(np.int64)
    my = (ar >= y1) & (ar < y2) if y1 < y2 else (ar > y2) & (ar <= y1)
    m2 = np.float32(dx) / np.float32(1 if dy == 0 else dy)
    xs = np.round(m2 * (ar - y1).astype(np.float32) + np.float32(x1)).astype(np.int64)
    ends = np.zeros((256, 256), np.float32)
    ends[ys[mx], ar[mx]] = 1.0
    ends[ar[my], xs[my]] = 1.0
    # the reference scatters masked-out lanes at index -1, which wraps
    if (~mx).any() or (~my).any():
        ends[255, 255] = 1.0
    ends[y1, x1] = 2.0
    ends[y2, x2] = 2.0
    return ends


# --------------------------------------------------------------------------
# Device program
# --------------------------------------------------------------------------
def build_nc(spc=8, b0_nonzero=False):
    """Build the per-core Bass program for `spc` samples (must be mult of 2)."""
    assert spc % 2 == 0
    nc = bacc.Bacc("TRN2", target_bir_lowering=False, debug=False,
                   num_devices=N_CORES)
    npair = spc // 2

    x0_d = nc.dram_tensor("x0", [npair, 128, 8192], F32R, kind="ExternalInput")
    w0_d = nc.dram_tensor("w0", [128, 64], F32R, kind="ExternalInput")
    w1_d = nc.dram_tensor("w1", [128, 16, 128], F32R, kind="ExternalInput")
    w2_d = nc.dram_tensor("w2", [128, 16, 256], F32R, kind="ExternalInput")
    w3_d = nc.dram_tensor("w3", [4, 2, 128, 16, 128], F32R, kind="ExternalInput")
    w4_d = nc.dram_tensor("w4", [4, 4, 128, 16, 128], F32R, kind="ExternalInput")
    w5_d = nc.dram_tensor("w5", [128, 4, 16], F32R, kind="ExternalInput")
    bn1_d = nc.dram_tensor("bn1", [128, 1, 2], F32, kind="ExternalInput")
    bn2_d = nc.dram_tensor("bn2", [128, 2, 2], F32, kind="ExternalInput")
    bn3_d = nc.dram_tensor("bn3", [128, 4, 2], F32, kind="ExternalInput")
    bn4_d = nc.dram_tensor("bn4", [128, 4, 2], F32, kind="ExternalInput")
    y_d = nc.dram_tensor("y", [1, spc, 256], F32, kind="ExternalOutput")
    h1_dram = nc.dram_tensor("h1tmp", [spc, 128, 4096], F32)

    # batch-stat denominators (full batch across all 8 cores)
    N1 = float(B * 64 * 64)
    N2 = float(B * 32 * 32)
    N3 = float(B * 16 * 16)
    N4 = float(B * 16 * 16)

    with TileContext(nc) as tc:
        import contextlib
        es = contextlib.ExitStack()
        with es:
            persist = es.enter_context(tc.tile_pool(name="persist", bufs=1))
            stats = es.enter_context(tc.tile_pool(name="stats", bufs=1))
            ccdram = es.enter_context(tc.tile_pool(name="ccdram", bufs=1, space="DRAM"))

            # ---- persistent small tensors ----
            w0_sb = persist.tile([128, 64], F32R)
            nc.sync.dma_start(out=w0_sb[:], in_=w0_d[:])
            w1_sb = persist.tile([128, 16, 128], F32R)
            nc.sync.dma_start(out=w1_sb[:], in_=w1_d[:])
            w5_sb = persist.tile([128, 4, 16], F32R)
            nc.sync.dma_start(out=w5_sb[:], in_=w5_d[:])
            bn_sb = {}
            for li, (d, nb) in {1: (bn1_d, 1), 2: (bn2_d, 2), 3: (bn3_d, 4), 4: (bn4_d, 4)}.items():
                t = persist.tile([128, nb, 2], F32, tag=f"bn{li}", name=f"bn{li}")
                nc.sync.dma_start(out=t[:], in_=d[:])
                bn_sb[li] = t
            eps_sb = persist.tile([128, 1], F32)
            nc.vector.memset(eps_sb[:], EPS)
            zero_sb = persist.tile([128, 288], F32)
            nc.gpsimd.memset(zero_sb[:], 0.0)

            def zf(dst):
                """Zero-fill an f32r AP via a rounding copy from the zero tile."""
                shp = list(dst.shape[1:])
                n = 1
                for d in shp:
                    n *= int(d)
                zv = zero_sb[:, 0:n]
                if len(shp) == 2:
                    zv = zv.rearrange("p (a b) -> p a b", a=int(shp[0]))
                elif len(shp) == 3:
                    zv = zv.rearrange("p (a b c) -> p a b c", a=int(shp[0]), b=int(shp[1]))
                nc.vector.tensor_copy(dst, zv)

            # per-layer BN scale/shift results [128, nb]
            scsh = {li: (persist.tile([128, nb], F32, tag=f"sc{li}", name=f"sc{li}"),
                         persist.tile([128, nb], F32, tag=f"sh{li}", name=f"sh{li}"))
                    for li, nb in {1: 1, 2: 2, 3: 4, 4: 4}.items()}

            # stat partial tiles
            part = {
                1: (stats.tile([128, 8 * spc], F32, tag="p1s", name="p1s"),
                    stats.tile([128, 8 * spc], F32, tag="p1q", name="p1q")),
                2: (stats.tile([128, 2, 2 * spc], F32, tag="p2s", name="p2s"),
                    stats.tile([128, 2, 2 * spc], F32, tag="p2q", name="p2q")),
                3: (stats.tile([128, 4, spc], F32, tag="p3s", name="p3s"),
                    stats.tile([128, 4, spc], F32, tag="p3q", name="p3q")),
                4: (stats.tile([128, 4, spc], F32, tag="p4s", name="p4s"),
                    stats.tile([128, 4, spc], F32, tag="p4q", name="p4q")),
            }

            def bn_reduce_allreduce(li, nb, n_elem):
                """partials -> AllReduce -> sc/sh [128, nb]."""
                ps, pq = part[li]
                cc_in = stats.tile([128, 2 * nb], F32, tag=f"ccin{li}", name=f"ccin{li}")
                for b in range(nb):
                    pss = ps[:] if nb == 1 else ps[:, b, :]
                    pqq = pq[:] if nb == 1 else pq[:, b, :]
                    nc.vector.tensor_reduce(out=cc_in[:, b:b + 1], in_=pss,
                                            axis=mybir.AxisListType.X, op=ADD)
                    nc.vector.tensor_reduce(out=cc_in[:, nb + b:nb + b + 1], in_=pqq,
                                            axis=mybir.AxisListType.X, op=ADD)
                din = ccdram.tile([128, 2 * nb], F32, tag=f"ccdin{li}", name=f"ccdin{li}")
                dout = ccdram.tile([128, 2 * nb], F32, tag=f"ccdout{li}", name=f"ccdout{li}")
                nc.gpsimd.dma_start(out=din[:], in_=cc_in[:])
                nc.gpsimd.collective_compute(
                    "AllReduce", ADD,
                    replica_groups=[list(range(N_CORES))],
                    ins=[din.opt()], outs=[dout.opt()],
                )
                g_sb = stats.tile([128, 2 * nb], F32, tag=f"ccg{li}", name=f"ccg{li}")
                nc.gpsimd.dma_start(out=g_sb[:], in_=dout[:])
                sc_t, sh_t = scsh[li]
                mean = stats.tile([128, nb], F32, tag=f"mean{li}", name=f"mean{li}")
                negm2 = stats.tile([128, nb], F32, tag=f"negm2{li}", name=f"negm2{li}")
                var = stats.tile([128, nb], F32, tag=f"var{li}", name=f"var{li}")
                for b in range(nb):
                    nc.vector.tensor_scalar_mul(mean[:, b:b + 1], g_sb[:, b:b + 1], 1.0 / n_elem)
                    nc.vector.scalar_tensor_tensor(out=negm2[:, b:b + 1], in0=mean[:, b:b + 1],
                                                   scalar=-1.0, in1=mean[:, b:b + 1],
                                                   op0=MULT, op1=MULT)
                    nc.vector.scalar_tensor_tensor(out=var[:, b:b + 1], in0=g_sb[:, nb + b:nb + b + 1],
                                                   scalar=1.0 / n_elem, in1=negm2[:, b:b + 1],
                                                   op0=MULT, op1=ADD)
                    # sd = sqrt(var + eps); rstd = 1/sd
                    nc.scalar.activation(out=var[:, b:b + 1], in_=var[:, b:b + 1],
                                         func=ACTF.Sqrt, bias=eps_sb[:], scale=1.0)
                    nc.vector.reciprocal(out=var[:, b:b + 1], in_=var[:, b:b + 1])
                    gcol = bn_sb[li][:, b, 0:1]
                    becol = bn_sb[li][:, b, 1:2]
                    nc.vector.tensor_mul(sc_t[:, b:b + 1], var[:, b:b + 1], gcol)
                    # sh = be - mean * sc
                    nc.vector.tensor_mul(mean[:, b:b + 1], mean[:, b:b + 1], sc_t[:, b:b + 1])
                    nc.vector.scalar_tensor_tensor(out=sh_t[:, b:b + 1], in0=mean[:, b:b + 1],
                                                   scalar=-1.0, in1=becol,
                                                   op0=MULT, op1=ADD)

            # ============================ PHASE A ============================
            with tc.tile_pool(name="pa_x", bufs=2) as pa_x, \
                 tc.tile_pool(name="pa_p1", bufs=1) as pa_p1, \
                 tc.tile_pool(name="pa_h1", bufs=2) as pa_h1, \
                 tc.tile_pool(name="pa_scr", bufs=2) as pa_scr, \
                 tc.tile_pool(name="ps_l0", bufs=2, space="PSUM") as ps_l0, \
                 tc.tile_pool(name="ps_l1", bufs=4, space="PSUM") as ps_l1:
                s1p, s1q = part[1]
                for pr in range(npair):
                    xp = pa_x.tile([128, 8192], F32R, tag="x0", name="xp")
                    for q in range(2):
                        nc.sync.dma_start(out=xp[:, 4096 * q:4096 * (q + 1)],
                                          in_=x0_d[pr, :, 4096 * q:4096 * (q + 1)])
                    p1t = pa_p1.tile([128, 16900], F32R, tag="p1", name="p1t")
                    pv = p1t.rearrange("p (h w) -> p h w", h=130)
                    zf(pv[:, 0, :])
                    zf(pv[:, 129, :])
                    zf(pv[:, 1:129, 0:130:129])
                    # ---- conv0: 2 samples packed (rows = K, cols = out half) ----
                    for c in range(32):
                        half = c // 16
                        poff = 64 * half
                        cc = c % 16
                        psA = ps_l0.tile([64, 512], F32, tag="l0a", name="psl0a")
                        psB = ps_l0.tile([64, 512], F32, tag="l0b", name="psl0b")
                        nc.tensor.matmul(psA[:],
                                         w0_sb[poff:poff + 32, :],
                                         xp[poff:poff + 32, 512 * cc:512 * (cc + 1)],
                                         start=True, stop=True,
                                         tile_position=(poff, 0))
                        nc.tensor.matmul(psB[:],
                                         w0_sb[poff + 32:poff + 64, :],
                                         xp[poff + 32:poff + 64, 512 * cc:512 * (cc + 1)],
                                         start=True, stop=True,
                                         tile_position=(poff + 32, 0))
                        for ps_t, plo in ((psA, 0), (psB, 64)):
                            dst = pv[plo:plo + 64, 4 * c + 1:4 * c + 5, 1:129]
                            nc.scalar.activation(out=dst,
                                                 in_=ps_t[:].rearrange("p (a b) -> p a b", a=4),
                                                 func=ACTF.Copy)
                            nc.vector.scalar_tensor_tensor(out=dst, in0=dst, scalar=0.2,
                                                           in1=dst, op0=MULT, op1=MAX)
                    # ---- conv1: 2 samples packed on partition halves ----
                    h1s = [pa_h1.tile([128, 4096], F32, tag="h1", name="h1s")
                           for _ in range(2)]
                    for grp in range(4):
                        pss = [[ps_l1.tile([128, 512], F32, tag="l1", name="psl1")
                                for _ in range(2)] for _ in range(2)]
                        for tap, (ky, kx) in enumerate(_TAPS):
                            for s in range(2):
                                lhsT = w1_sb[64 * s:64 * s + 64, tap, :]
                                for ch in range(2):
                                    chunk = grp * 2 + ch
                                    rhs = pv[64 * s:64 * s + 64,
                                             ky + 16 * chunk:ky + 16 * chunk + 15:2,
                                             kx:kx + 127:2]
                                    nc.tensor.matmul(pss[s][ch][:], lhsT, rhs,
                                                     start=(tap == 0), stop=(tap == 15),
                                                     tile_position=(64 * s, 0))
                        for s in range(2):
                            sg = 2 * pr + s
                            for ch in range(2):
                                chunk = grp * 2 + ch
                                slot = sg * 8 + chunk
                                nc.scalar.activation(
                                    out=h1s[s][:, 512 * chunk:512 * (chunk + 1)],
                                    in_=pss[s][ch][:], func=ACTF.Copy,
                                    accum_out=s1p[:, slot:slot + 1])
                                scr = pa_scr.tile([128, 512], F32, tag="scr", name="scr")
                                nc.scalar.activation(
                                    out=scr[:], in_=pss[s][ch][:], func=ACTF.Square,
                                    accum_out=s1q[:, slot:slot + 1])
                    for s in range(2):
                        sg = 2 * pr + s
                        nc.sync.dma_start(out=h1_dram[sg], in_=h1s[s][:])

            bn_reduce_allreduce(1, 1, N1)

            # ============================ PHASE B ============================
            sc1, sh1 = scsh[1]
            ph2 = tc.alloc_tile_pool(name="ph2", bufs=1)
            h2 = [ph2.tile([128, spc, 1156], F32R, tag=f"h2_{kb}", name=f"h2_{kb}")
                  for kb in range(2)]
            h2v = [h.rearrange("p s (h w) -> p s h w", h=34) for h in h2]
            for kb in range(2):
                zf(h2v[kb][:, :, 0, :])
                zf(h2v[kb][:, :, 33, :])
                zf(h2v[kb][:, :, 1:33, 0])
                zf(h2v[kb][:, :, 1:33, 33])
            with tc.tile_pool(name="pb_in", bufs=2) as pb_in, \
                 tc.tile_pool(name="pb_p2", bufs=2) as pb_p2, \
                 tc.tile_pool(name="pb_w", bufs=1) as pb_w, \
                 tc.tile_pool(name="pb_scr", bufs=2) as pb_scr, \
                 tc.tile_pool(name="ps_l2", bufs=8, space="PSUM") as ps_l2:
                w2_sb = pb_w.tile([128, 16, 256], F32R)
                nc.sync.dma_start(out=w2_sb[:], in_=w2_d[:])
                s2p, s2q = part[2]
                for s in range(spc):
                    lb = pb_in.tile([128, 4096], F32, tag="lb", name="lb")
                    nc.sync.dma_start(out=lb[:], in_=h1_dram[s])
                    p2t = pb_p2.tile([128, 4356], F32R, tag="p2", name="p2t")
                    p2 = p2t.rearrange("p (h w) -> p h w", h=66)
                    zf(p2[:, 0, :])
                    zf(p2[:, 65, :])
                    zf(p2[:, 1:65, 0:66:65])
                    interior = p2[:, 1:65, 1:65]
                    nc.vector.tensor_scalar(out=interior,
                                            in0=lb[:].rearrange("p (h w) -> p h w", h=64),
                                            scalar1=sc1[:, 0:1], scalar2=sh1[:, 0:1],
                                            op0=MULT, op1=ADD)
                    nc.vector.scalar_tensor_tensor(out=interior, in0=interior, scalar=0.2,
                                                   in1=interior, op0=MULT, op1=MAX)
                    for mt in range(2):
                        pss = [ps_l2.tile([128, 512], F32, tag="l2", name="psl2") for _ in range(2)]
                        for tap, (ky, kx) in enumerate(_TAPS):
                            lhsT = w2_sb[:, tap, 128 * mt:128 * mt + 128]
                            for ch in range(2):
                                rhs = p2[:, ky + 32 * ch:ky + 32 * ch + 31:2, kx:kx + 63:2]
                                nc.tensor.matmul(pss[ch][:], lhsT, rhs,
                                                 start=(tap == 0), stop=(tap == 15))
                        for ch in range(2):
                            slot = s * 2 + ch
                            dst = h2v[mt][:, s, 16 * ch + 1:16 * ch + 17, 1:33]
                            nc.scalar.activation(out=dst,
                                                 in_=pss[ch][:].rearrange("p (a b) -> p a b", a=16),
                                                 func=ACTF.Copy,
                                                 accum_out=s2p[:, mt, slot:slot + 1])
                            scr = pb_scr.tile([128, 512], F32, tag="scr", name="scr")
                            nc.scalar.activation(out=scr[:], in_=pss[ch][:], func=ACTF.Square,
                                                 accum_out=s2q[:, mt, slot:slot + 1])

            bn_reduce_allreduce(2, 2, N2)

            # ============================ PHASE C ============================
            sc2, sh2 = scsh[2]
            for kb in range(2):
                for s in range(spc):
                    inter = h2v[kb][:, s, 1:33, 1:33]
                    nc.vector.tensor_scalar(out=inter, in0=inter, scalar1=sc2[:, kb:kb + 1],
                                            scalar2=sh2[:, kb:kb + 1], op0=MULT, op1=ADD)
                    nc.vector.scalar_tensor_tensor(out=inter, in0=inter, scalar=0.2,
                                                   in1=inter, op0=MULT, op1=MAX)
            ph3 = tc.alloc_tile_pool(name="ph3", bufs=1, side="right")
            h3 = [ph3.tile([128, spc, 361], F32R, tag=f"h3_{kb}", name=f"h3_{kb}")
                  for kb in range(4)]
            h3v = [h.rearrange("p s (h w) -> p s h w", h=19) for h in h3]
            for kb in range(4):
                zf(h3v[kb][:, :, 0, :])
                zf(h3v[kb][:, :, 17, :])
                zf(h3v[kb][:, :, 18, :])
                zf(h3v[kb][:, :, 1:17, 0])
                zf(h3v[kb][:, :, 1:17, 17])
                zf(h3v[kb][:, :, 1:17, 18])
            with tc.tile_pool(name="pc_w", bufs=4) as pc_w, \
                 tc.tile_pool(name="pc_scr", bufs=2) as pc_scr, \
                 tc.tile_pool(name="ps_l3", bufs=8, space="PSUM") as ps_l3:
                s3p, s3q = part[3]
                for mt in range(4):
                    w3t = [pc_w.tile([128, 16, 128], F32R, tag="w3", name="w3t") for _ in range(2)]
                    for kb in range(2):
                        nc.sync.dma_start(out=w3t[kb][:], in_=w3_d[mt, kb])
                    pss = [ps_l3.tile([128, 256], F32, tag="l3", name="psl3")
                           for _ in range(spc)]
                    for kb in range(2):
                        for tap, (ky, kx) in enumerate(_TAPS):
                            lhsT = w3t[kb][:, tap, :]
                            for s in range(spc):
                                rhs = h2v[kb][:, s, ky:ky + 31:2, kx:kx + 31:2]
                                nc.tensor.matmul(pss[s][:], lhsT, rhs,
                                                 start=(kb == 0 and tap == 0),
                                                 stop=(kb == 1 and tap == 15))
                    for s in range(spc):
                        dst = h3v[mt][:, s, 1:17, 1:17]
                        nc.scalar.activation(out=dst,
                                             in_=pss[s][:].rearrange("p (a b) -> p a b", a=16),
                                             func=ACTF.Copy, accum_out=s3p[:, mt, s:s + 1])
                        scr = pc_scr.tile([128, 512], F32, tag="scr", name="scr")
                        nc.scalar.activation(out=scr[:, 0:256], in_=pss[s][:], func=ACTF.Square,
                                             accum_out=s3q[:, mt, s:s + 1])
            ph2.release()

            bn_reduce_allreduce(3, 4, N3)

            # ============================ PHASE D ============================
            sc3, sh3 = scsh[3]
            for kb in range(4):
                for s in range(spc):
                    inter = h3v[kb][:, s, 1:17, 1:17]
                    nc.vector.tensor_scalar(out=inter, in0=inter, scalar1=sc3[:, kb:kb + 1],
                                            scalar2=sh3[:, kb:kb + 1], op0=MULT, op1=ADD)
                    nc.vector.scalar_tensor_tensor(out=inter, in0=inter, scalar=0.2,
                                                   in1=inter, op0=MULT, op1=MAX)
            ph4 = tc.alloc_tile_pool(name="ph4", bufs=1)
            h4 = [ph4.tile([128, spc, 361], F32R, tag=f"h4_{kb}", name=f"h4_{kb}")
                  for kb in range(4)]
            h4v = [h.rearrange("p s (h w) -> p s h w", h=19) for h in h4]
            for kb in range(4):
                zf(h4v[kb][:, :, 0, :])
                zf(h4v[kb][:, :, 17, :])
                zf(h4v[kb][:, :, 18, :])
                zf(h4v[kb][:, :, 1:17, 0])
                zf(h4v[kb][:, :, 1:17, 17])
                zf(h4v[kb][:, :, 1:17, 18])
            with tc.tile_pool(name="pd_w", bufs=6) as pd_w, \
                 tc.tile_pool(name="pd_scr", bufs=2) as pd_scr, \
                 tc.tile_pool(name="ps_l4", bufs=8, space="PSUM") as ps_l4:
                s4p, s4q = part[4]
                for mt in range(4):
                    w4t = [pd_w.tile([128, 16, 128], F32R, tag="w4", name="w4t") for _ in range(4)]
                    for kb in range(4):
                        nc.sync.dma_start(out=w4t[kb][:], in_=w4_d[mt, kb])
                    pss = [ps_l4.tile([128, 256], F32, tag="l4", name="psl4")
                           for _ in range(spc)]
                    for kb in range(4):
                        for tap, (ky, kx) in enumerate(_TAPS):
                            lhsT = w4t[kb][:, tap, :]
                            for s in range(spc):
                                rhs = h3v[kb][:, s, ky:ky + 16, kx:kx + 16]
                                nc.tensor.matmul(pss[s][:], lhsT, rhs,
                                                 start=(kb == 0 and tap == 0),
                                                 stop=(kb == 3 and tap == 15))
                    for s in range(spc):
                        dst = h4v[mt][:, s, 1:17, 1:17]
                        nc.scalar.activation(out=dst,
                                             in_=pss[s][:].rearrange("p (a b) -> p a b", a=16),
                                             func=ACTF.Copy, accum_out=s4p[:, mt, s:s + 1])
                        scr = pd_scr.tile([128, 512], F32, tag="scr", name="scr")
                        nc.scalar.activation(out=scr[:, 0:256], in_=pss[s][:], func=ACTF.Square,
                                             accum_out=s4q[:, mt, s:s + 1])
            ph3.release()

            bn_reduce_allreduce(4, 4, N4)

            # ============================ PHASE E ============================
            sc4, sh4 = scsh[4]
            for kb in range(4):
                for s in range(spc):
                    inter = h4v[kb][:, s, 1:17, 1:17]
                    nc.vector.tensor_scalar(out=inter, in0=inter, scalar1=sc4[:, kb:kb + 1],
                                            scalar2=sh4[:, kb:kb + 1], op0=MULT, op1=ADD)
                    nc.vector.scalar_tensor_tensor(out=inter, in0=inter, scalar=0.2,
                                                   in1=inter, op0=MULT, op1=MAX)
            with tc.tile_pool(name="pe_out", bufs=1) as pe_out, \
                 tc.tile_pool(name="ps_l5", bufs=8, space="PSUM") as ps_l5:
                ys = pe_out.tile([1, spc, 256], F32)
                pse = [ps_l5.tile([1, 256], F32, tag="l5", name="psl5") for s in range(spc)]
                for kb in range(4):
                    for tap, (ky, kx) in enumerate(_TAPS):
                        lhsT = w5_sb[:, kb, tap:tap + 1]
                        for s in range(spc):
                            rhs = h4v[kb][:, s, ky:ky + 16, kx:kx + 16]
                            nc.tensor.matmul(pse[s][:], lhsT, rhs,
                                             start=(kb == 0 and tap == 0),
                                             stop=(kb == 3 and tap == 15))
                for s in range(spc):
                    nc.scalar.activation(out=ys[:, s, :], in_=pse[s][:], func=ACTF.Copy)
                nc.sync.dma_start(out=y_d[:], in_=ys[:])
            ph4.release()
    nc.compile()
    return nc


# --------------------------------------------------------------------------
# Host-side input prep
# --------------------------------------------------------------------------
def _prep_inputs(x, params, spc=8, cores=N_CORES):
    x = np.ascontiguousarray(np.asarray(x, np.float32))
    p = {k: np.asarray(v, np.float32) for k, v in params.items()}

    ends = np.stack([_rasterize_host(x[i, 1]) for i in range(x.shape[0])])
    xin = np.zeros((x.shape[0], 2, 258, 258), np.float32)
    xin[:, 0, 1:257, 1:257] = x[:, 0]
    xin[:, 1, 1:257, 1:257] = ends
    # im2col rows ordered (ky, kx, ci)
    cols = np.empty((x.shape[0], 4, 4, 2, 128, 128), np.float32)
    for ky in range(4):
        for kx in range(4):
            cols[:, ky, kx] = xin[:, :, ky:ky + 255:2, kx:kx + 255:2]
    imcol = cols.reshape(x.shape[0], 32, 16384)

    w0 = np.tile(p["w0"].transpose(2, 3, 1, 0).reshape(32, 64), (4, 1))
    w1 = np.tile(p["w1"].transpose(1, 2, 3, 0).reshape(64, 16, 128), (2, 1, 1))
    w2 = p["w2"].transpose(1, 2, 3, 0).reshape(128, 16, 256)
    w3 = (p["w3"].transpose(1, 2, 3, 0).reshape(2, 128, 16, 4, 128)
          .transpose(3, 0, 1, 2, 4))
    w4 = (p["w4"].transpose(1, 2, 3, 0).reshape(4, 128, 16, 4, 128)
          .transpose(3, 0, 1, 2, 4))
    w5 = p["w5"].transpose(1, 2, 3, 0).reshape(4, 128, 16).transpose(1, 0, 2)

    def bn_pack(li, nb):
        g = p[f"g{li}"].reshape(nb, 128)
        be = p[f"be{li}"].reshape(nb, 128)
        t = np.empty((128, nb, 2), np.float32)
        t[:, :, 0] = g.T
        t[:, :, 1] = be.T
        return t

    # NOTE: conv biases b1..b4 cancel inside training-mode BN (they shift the
    # mean by exactly b); b5 is added on the host after the gather; b0 is zero
    # in this problem's setup (asserted in kernel()).
    shared = {
        "w0": np.ascontiguousarray(w0), "w1": np.ascontiguousarray(w1),
        "w2": np.ascontiguousarray(w2), "w3": np.ascontiguousarray(w3),
        "w4": np.ascontiguousarray(w4), "w5": np.ascontiguousarray(w5),
        "bn1": bn_pack(1, 1), "bn2": bn_pack(2, 2),
        "bn3": bn_pack(3, 4), "bn4": bn_pack(4, 4),
    }
    in_maps = []
    for c in range(cores):
        sm = imcol[c * spc:(c + 1) * spc]  # [spc, 32, 16384]
        x0 = np.empty((spc // 2, 128, 8192), np.float32)
        for pr in range(spc // 2):
            x0[pr, 0:32] = sm[2 * pr, :, :8192]
            x0[pr, 32:64] = sm[2 * pr + 1, :, :8192]
            x0[pr, 64:96] = sm[2 * pr, :, 8192:]
            x0[pr, 96:128] = sm[2 * pr + 1, :, 8192:]
        m = dict(shared)
        m["x0"] = np.ascontiguousarray(x0)
        in_maps.append(m)
    return in_maps, p


_NC_CACHE = {}


def kernel(x, params):
    global LAST_EXEC_NS
    in_maps, p = _prep_inputs(x, params, spc=SPC)
    assert not np.any(p["b0"]), "conv0 bias path not implemented"
    key = (SPC,)
    if key not in _NC_CACHE:
        _NC_CACHE[key] = build_nc(spc=SPC)
    nc = _NC_CACHE[key]
    res = run_bass_kernel_spmd(
        nc, in_maps, list(range(N_CORES)),
        trace=TRACE, tmpdir=TRACE_TMPDIR,
    )
    LAST_EXEC_NS = res.exec_time_ns
    out = np.concatenate([res.results[c]["y"].reshape(SPC, 256) for c in range(N_CORES)], axis=0)
    out = out.reshape(B, 1, 16, 16) + p["b5"].reshape(1, 1, 1, 1)
    return out.astype(np.float32)


# revision 12
# speedup vs baseline: 2.1439x; 1.0032x over previous
"""Trainium2 Bass kernel for nn_Critic (6-layer conv critic with line rasterization
and training-mode BatchNorm), data-parallel over 8 NeuronCores.

Strategy:
  - Host: rasterize the two endpoint pixels per sample into the line map
    (bit-exact replication of the jax reference, including its index -1 ->
    (255,255) wraparound), then build a 32-row im2col layout for conv0
    (K = 2ch x 16 taps) so the K=2 first conv becomes a K=32 matmul.
  - Device (per core, 8 samples): 5 phases.
      A: conv0 (4-sample tile_position row/col packing) + lrelu fused into
         PSUM eviction -> padded conv1 input; conv1 (2-sample row packing);
         per-channel sum/sumsq accumulated during eviction; h1 spilled to HBM.
      AllReduce #1 of BN1 stats (8 cores).
      B: load h1, apply BN1 affine + lrelu, pad; conv2; h2 stays in SBUF.
      AllReduce #2.  C: BN2+lrelu in place; conv3 (weights streamed from HBM).
      AllReduce #3.  D: BN3+lrelu; conv4 (weights streamed).
      AllReduce #4.  E: BN4+lrelu; conv5 (C_out=1).
  - Convs are computed as 16 shifted strided-AP matmuls accumulating in PSUM
    (channels on partitions); stride-2 selection is folded into the rhs AP.
"""

import os
import numpy as np

import concourse.bacc as bacc
import concourse.mybir as mybir
from concourse.tile import TileContext
from concourse.bass_utils import run_bass_kernel_spmd

F32 = mybir.dt.float32
F32R = mybir.dt.float32r
MULT = mybir.AluOpType.mult
ADD = mybir.AluOpType.add
MAX = mybir.AluOpType.max
ACTF = mybir.ActivationFunctionType

N_CORES = 8
B = 64
H = W = 256
EPS = 1e-3

# knobs for the test harness (module-level; harness defaults are fine)
TRACE = False
TRACE_TMPDIR = None
LAST_EXEC_NS = None
SPC = 8            # samples per core
SIM = False        # build-only / simulated run (set by sim harness)

_TAPS = [(ky, kx) for ky in range(4) for kx in range(4)]


# --------------------------------------------------------------------------
# Host-side rasterization (bit-exact vs the jax reference on f32 math)
# --------------------------------------------------------------------------
def _rasterize_host(ch):
    flat = ch.reshape(-1)
    idx = np.nonzero(flat == 2.0)[0]
    i0, i1 = int(idx[0]), int(idx[1])
    y1, x1 = i0 // W, i0 % W
    y2, x2 = i1 // W, i1 % W
    ar = np.arange(256)
    dx, dy = x2 - x1, y2 - y1
    mx = (ar >= x1) & (ar < x2) if x1 < x2 else (ar > x2) & (ar <= x1)
    m1 = np.float32(dy) / np.float32(1 if dx == 0 else dx)
    ys = np.round(m1 * (ar - x1).astype(np.float32) + np.float32(y1)).astype(np.int64)
    my = (ar >= y1) & (ar < y2) if y1 < y2 else (ar > y2) & (ar <= y1)
    m2 = np.float32(dx) / np.float32(1 if dy == 0 else dy)
    xs = np.round(m2 * (ar - y1).astype(np.float32) + np.float32(x1)).astype(np.int64)
    ends = np.zeros((256, 256), np.float32)
    ends[ys[mx], ar[mx]] = 1.0
    ends[ar[my], xs[my]] = 1.0
    # the reference scatters masked-out lanes at index -1, which wraps
    if (~mx).any() or (~my).any():
        ends[255, 255] = 1.0
    ends[y1, x1] = 2.0
    ends[y2, x2] = 2.0
    return ends


# --------------------------------------------------------------------------
# Device program
# --------------------------------------------------------------------------
def build_nc(spc=8, b0_nonzero=False):
    """Build the per-core Bass program for `spc` samples (must be mult of 2)."""
    assert spc % 2 == 0
    nc = bacc.Bacc("TRN2", target_bir_lowering=False, debug=False,
                   num_devices=N_CORES)
    npair = spc // 2

    x0_d = nc.dram_tensor("x0", [npair, 128, 8192], F32R, kind="ExternalInput")
    w0_d = nc.dram_tensor("w0", [128, 64], F32R, kind="ExternalInput")
    w1_d = nc.dram_tensor("w1", [128, 16, 128], F32R, kind="ExternalInput")
    w2_d = nc.dram_tensor("w2", [128, 16, 256], F32R, kind="ExternalInput")
    w3_d = nc.dram_tensor("w3", [4, 2, 128, 16, 128], F32R, kind="ExternalInput")
    w4_d = nc.dram_tensor("w4", [4, 4, 128, 16, 128], F32R, kind="ExternalInput")
    w5_d = nc.dram_tensor("w5", [128, 4, 16], F32R, kind="ExternalInput")
    bn1_d = nc.dram_tensor("bn1", [128, 1, 2], F32, kind="ExternalInput")
    bn2_d = nc.dram_tensor("bn2", [128, 2, 2], F32, kind="ExternalInput")
    bn3_d = nc.dram_tensor("bn3", [128, 4, 2], F32, kind="ExternalInput")
    bn4_d = nc.dram_tensor("bn4", [128, 4, 2], F32, kind="ExternalInput")
    y_d = nc.dram_tensor("y", [1, spc, 256], F32, kind="ExternalOutput")
    h1_dram = nc.dram_tensor("h1tmp", [spc, 128, 4096], F32)

    # batch-stat denominators (full batch across all 8 cores)
    N1 = float(B * 64 * 64)
    N2 = float(B * 32 * 32)
    N3 = float(B * 16 * 16)
    N4 = float(B * 16 * 16)

    with TileContext(nc) as tc:
        import contextlib
        es = contextlib.ExitStack()
        with es:
            persist = es.enter_context(tc.tile_pool(name="persist", bufs=1))
            stats = es.enter_context(tc.tile_pool(name="stats", bufs=1))
            ccdram = es.enter_context(tc.tile_pool(name="ccdram", bufs=1, space="DRAM"))

            # ---- persistent small tensors ----
            w0_sb = persist.tile([128, 64], F32R)
            nc.sync.dma_start(out=w0_sb[:], in_=w0_d[:])
            w1_sb = persist.tile([128, 16, 128], F32R)
            nc.sync.dma_start(out=w1_sb[:], in_=w1_d[:])
            w5_sb = persist.tile([128, 4, 16], F32R)
            nc.sync.dma_start(out=w5_sb[:], in_=w5_d[:])
            bn_sb = {}
            for li, (d, nb) in {1: (bn1_d, 1), 2: (bn2_d, 2), 3: (bn3_d, 4), 4: (bn4_d, 4)}.items():
                t = persist.tile([128, nb, 2], F32, tag=f"bn{li}", name=f"bn{li}")
                nc.sync.dma_start(out=t[:], in_=d[:])
                bn_sb[li] = t
            eps_sb = persist.tile([128, 1], F32)
            nc.vector.memset(eps_sb[:], EPS)
            zero_sb = persist.tile([128, 288], F32)
            nc.gpsimd.memset(zero_sb[:], 0.0)

            def zf(dst):
                """Zero-fill an f32r AP via a rounding copy from the zero tile."""
                shp = list(dst.shape[1:])
                n = 1
                for d in shp:
                    n *= int(d)
                zv = zero_sb[:, 0:n]
                if len(shp) == 2:
                    zv = zv.rearrange("p (a b) -> p a b", a=int(shp[0]))
                elif len(shp) == 3:
                    zv = zv.rearrange("p (a b c) -> p a b c", a=int(shp[0]), b=int(shp[1]))
                nc.vector.tensor_copy(dst, zv)

            # per-layer BN scale/shift results [128, nb]
            scsh = {li: (persist.tile([128, nb], F32, tag=f"sc{li}", name=f"sc{li}"),
                         persist.tile([128, nb], F32, tag=f"sh{li}", name=f"sh{li}"))
                    for li, nb in {1: 1, 2: 2, 3: 4, 4: 4}.items()}

            # stat partial tiles
            part = {
                1: (stats.tile([128, 8 * spc], F32, tag="p1s", name="p1s"),
                    stats.tile([128, 8 * spc], F32, tag="p1q", name="p1q")),
                2: (stats.tile([128, 2, 2 * spc], F32, tag="p2s", name="p2s"),
                    stats.tile([128, 2, 2 * spc], F32, tag="p2q", name="p2q")),
                3: (stats.tile([128, 4, spc], F32, tag="p3s", name="p3s"),
                    stats.tile([128, 4, spc], F32, tag="p3q", name="p3q")),
                4: (stats.tile([128, 4, spc], F32, tag="p4s", name="p4s"),
                    stats.tile([128, 4, spc], F32, tag="p4q", name="p4q")),
            }

            def bn_reduce_allreduce(li, nb, n_elem):
                """partials -> AllReduce -> sc/sh [128, nb]."""
                ps, pq = part[li]
                cc_in = stats.tile([128, 2 * nb], F32, tag=f"ccin{li}", name=f"ccin{li}")
                for b in range(nb):
                    pss = ps[:] if nb == 1 else ps[:, b, :]
                    pqq = pq[:] if nb == 1 else pq[:, b, :]
                    nc.vector.tensor_reduce(out=cc_in[:, b:b + 1], in_=pss,
                                            axis=mybir.AxisListType.X, op=ADD)
                    nc.vector.tensor_reduce(out=cc_in[:, nb + b:nb + b + 1], in_=pqq,
                                            axis=mybir.AxisListType.X, op=ADD)
                din = ccdram.tile([128, 2 * nb], F32, tag=f"ccdin{li}", name=f"ccdin{li}")
                dout = ccdram.tile([128, 2 * nb], F32, tag=f"ccdout{li}", name=f"ccdout{li}")
                nc.gpsimd.dma_start(out=din[:], in_=cc_in[:])
                nc.gpsimd.collective_compute(
                    "AllReduce", ADD,
                    replica_groups=[list(range(N_CORES))],
                    ins=[din.opt()], outs=[dout.opt()],
                )
                g_sb = stats.tile([128, 2 * nb], F32, tag=f"ccg{li}", name=f"ccg{li}")
                nc.gpsimd.dma_start(out=g_sb[:], in_=dout[:])
                sc_t, sh_t = scsh[li]
                mean = stats.tile([128, nb], F32, tag=f"mean{li}", name=f"mean{li}")
                negm2 = stats.tile([128, nb], F32, tag=f"negm2{li}", name=f"negm2{li}")
                var = stats.tile([128, nb], F32, tag=f"var{li}", name=f"var{li}")
                for b in range(nb):
                    nc.vector.tensor_scalar_mul(mean[:, b:b + 1], g_sb[:, b:b + 1], 1.0 / n_elem)
                    nc.vector.scalar_tensor_tensor(out=negm2[:, b:b + 1], in0=mean[:, b:b + 1],
                                                   scalar=-1.0, in1=mean[:, b:b + 1],
                                                   op0=MULT, op1=MULT)
                    nc.vector.scalar_tensor_tensor(out=var[:, b:b + 1], in0=g_sb[:, nb + b:nb + b + 1],
                                                   scalar=1.0 / n_elem, in1=negm2[:, b:b + 1],
                                                   op0=MULT, op1=ADD)
                    # sd = sqrt(var + eps); rstd = 1/sd
                    nc.scalar.activation(out=var[:, b:b + 1], in_=var[:, b:b + 1],
                                         func=ACTF.Sqrt, bias=eps_sb[:], scale=1.0)
                    nc.vector.reciprocal(out=var[:, b:b + 1], in_=var[:, b:b + 1])
                    gcol = bn_sb[li][:, b, 0:1]
                    becol = bn_sb[li][:, b, 1:2]
                    nc.vector.tensor_mul(sc_t[:, b:b + 1], var[:, b:b + 1], gcol)
                    # sh = be - mean * sc
                    nc.vector.tensor_mul(mean[:, b:b + 1], mean[:, b:b + 1], sc_t[:, b:b + 1])
                    nc.vector.scalar_tensor_tensor(out=sh_t[:, b:b + 1], in0=mean[:, b:b + 1],
                                                   scalar=-1.0, in1=becol,
                                                   op0=MULT, op1=ADD)

            # ============================ PHASE A ============================
            with tc.tile_pool(name="pa_x", bufs=2) as pa_x, \
                 tc.tile_pool(name="pa_p1", bufs=1) as pa_p1, \
                 tc.tile_pool(name="pa_h1", bufs=2) as pa_h1, \
                 tc.tile_pool(name="pa_scr", bufs=2) as pa_scr, \
                 tc.tile_pool(name="ps_l0", bufs=2, space="PSUM") as ps_l0, \
                 tc.tile_pool(name="ps_l1", bufs=4, space="PSUM") as ps_l1:
                s1p, s1q = part[1]
                for pr in range(npair):
                    xp = pa_x.tile([128, 8192], F32R, tag="x0", name="xp")
                    for q in range(2):
                        nc.sync.dma_start(out=xp[:, 4096 * q:4096 * (q + 1)],
                                          in_=x0_d[pr, :, 4096 * q:4096 * (q + 1)])
                    p1t = pa_p1.tile([128, 16900], F32R, tag="p1", name="p1t")
                    pv = p1t.rearrange("p (h w) -> p h w", h=130)
                    zf(pv[:, 0, :])
                    zf(pv[:, 129, :])
                    zf(pv[:, 1:129, 0:130:129])
                    # ---- conv0: 2 samples packed (rows = K, cols = out half) ----
                    for c in range(32):
                        half = c // 16
                        poff = 64 * half
                        cc = c % 16
                        psA = ps_l0.tile([64, 512], F32, tag="l0a", name="psl0a")
                        psB = ps_l0.tile([64, 512], F32, tag="l0b", name="psl0b")
                        nc.tensor.matmul(psA[:],
                                         w0_sb[poff:poff + 32, :],
                                         xp[poff:poff + 32, 512 * cc:512 * (cc + 1)],
                                         start=True, stop=True,
                                         tile_position=(poff, 0))
                        nc.tensor.matmul(psB[:],
                                         w0_sb[poff + 32:poff + 64, :],
                                         xp[poff + 32:poff + 64, 512 * cc:512 * (cc + 1)],
                                         start=True, stop=True,
                                         tile_position=(poff + 32, 0))
                        for ps_t, plo in ((psA, 0), (psB, 64)):
                            dst = pv[plo:plo + 64, 4 * c + 1:4 * c + 5, 1:129]
                            nc.scalar.activation(out=dst,
                                                 in_=ps_t[:].rearrange("p (a b) -> p a b", a=4),
                                                 func=ACTF.Copy)
                            nc.vector.scalar_tensor_tensor(out=dst, in0=dst, scalar=0.2,
                                                           in1=dst, op0=MULT, op1=MAX)
                    # ---- conv1: 2 samples packed on partition halves ----
                    h1s = [pa_h1.tile([128, 4096], F32, tag="h1", name="h1s")
                           for _ in range(2)]
                    for grp in range(4):
                        pss = [[ps_l1.tile([128, 512], F32, tag="l1", name="psl1")
                                for _ in range(2)] for _ in range(2)]
                        for tap, (ky, kx) in enumerate(_TAPS):
                            for s in range(2):
                                lhsT = w1_sb[64 * s:64 * s + 64, tap, :]
                                for ch in range(2):
                                    chunk = grp * 2 + ch
                                    rhs = pv[64 * s:64 * s + 64,
                                             ky + 16 * chunk:ky + 16 * chunk + 15:2,
                                             kx:kx + 127:2]
                                    nc.tensor.matmul(pss[s][ch][:], lhsT, rhs,
                                                     start=(tap == 0), stop=(tap == 15),
                                                     tile_position=(64 * s, 0))
                        for s in range(2):
                            sg = 2 * pr + s
                            for ch in range(2):
                                chunk = grp * 2 + ch
                                slot = sg * 8 + chunk
                                nc.scalar.activation(
                                    out=h1s[s][:, 512 * chunk:512 * (chunk + 1)],
                                    in_=pss[s][ch][:], func=ACTF.Copy,
                                    accum_out=s1p[:, slot:slot + 1])
                                scr = pa_scr.tile([128, 512], F32, tag="scr", name="scr")
                                nc.scalar.activation(
                                    out=scr[:], in_=pss[s][ch][:], func=ACTF.Square,
                                    accum_out=s1q[:, slot:slot + 1])
                    for s in range(2):
                        sg = 2 * pr + s
                        nc.sync.dma_start(out=h1_dram[sg], in_=h1s[s][:])

            bn_reduce_allreduce(1, 1, N1)

            # ============================ PHASE B ============================
            sc1, sh1 = scsh[1]
            ph2 = tc.alloc_tile_pool(name="ph2", bufs=1)
            h2 = [ph2.tile([128, spc, 1156], F32R, tag=f"h2_{kb}", name=f"h2_{kb}")
                  for kb in range(2)]
            h2v = [h.rearrange("p s (h w) -> p s h w", h=34) for h in h2]
            for kb in range(2):
                zf(h2v[kb][:, :, 0, :])
                zf(h2v[kb][:, :, 33, :])
                zf(h2v[kb][:, :, 1:33, 0])
                zf(h2v[kb][:, :, 1:33, 33])
            with tc.tile_pool(name="pb_in", bufs=2) as pb_in, \
                 tc.tile_pool(name="pb_p2", bufs=2) as pb_p2, \
                 tc.tile_pool(name="pb_w", bufs=1) as pb_w, \
                 tc.tile_pool(name="pb_scr", bufs=2) as pb_scr, \
                 tc.tile_pool(name="ps_l2", bufs=8, space="PSUM") as ps_l2:
                w2_sb = pb_w.tile([128, 16, 256], F32R)
                nc.sync.dma_start(out=w2_sb[:], in_=w2_d[:])
                s2p, s2q = part[2]
                for s in range(spc):
                    lb = pb_in.tile([128, 4096], F32, tag="lb", name="lb")
                    nc.sync.dma_start(out=lb[:], in_=h1_dram[s])
                    p2t = pb_p2.tile([128, 4356], F32R, tag="p2", name="p2t")
                    p2 = p2t.rearrange("p (h w) -> p h w", h=66)
                    zf(p2[:, 0, :])
                    zf(p2[:, 65, :])
                    zf(p2[:, 1:65, 0:66:65])
                    interior = p2[:, 1:65, 1:65]
                    nc.vector.tensor_scalar(out=interior,
                                            in0=lb[:].rearrange("p (h w) -> p h w", h=64),
                                            scalar1=sc1[:, 0:1], scalar2=sh1[:, 0:1],
                                            op0=MULT, op1=ADD)
                    nc.vector.scalar_tensor_tensor(out=interior, in0=interior, scalar=0.2,
                                                   in1=interior, op0=MULT, op1=MAX)
                    for mt in range(2):
                        pss = [ps_l2.tile([128, 512], F32, tag="l2", name="psl2") for _ in range(2)]
                        for tap, (ky, kx) in enumerate(_TAPS):
                            lhsT = w2_sb[:, tap, 128 * mt:128 * mt + 128]
                            for ch in range(2):
                                rhs = p2[:, ky + 32 * ch:ky + 32 * ch + 31:2, kx:kx + 63:2]
                                nc.tensor.matmul(pss[ch][:], lhsT, rhs,
                                                 start=(tap == 0), stop=(tap == 15))
                        for ch in range(2):
                            slot = s * 2 + ch
                            dst = h2v[mt][:, s, 16 * ch + 1:16 * ch + 17, 1:33]
                            nc.scalar.activation(out=dst,
                                                 in_=pss[ch][:].rearrange("p (a b) -> p a b", a=16),
                                                 func=ACTF.Copy,
                                                 accum_out=s2p[:, mt, slot:slot + 1])
                            scr = pb_scr.tile([128, 512], F32, tag="scr", name="scr")
                            nc.scalar.activation(out=scr[:], in_=pss[ch][:], func=ACTF.Square,
                                                 accum_out=s2q[:, mt, slot:slot + 1])

            bn_reduce_allreduce(2, 2, N2)

            # ============================ PHASE C ============================
            sc2, sh2 = scsh[2]
            for kb in range(2):
                for s in range(spc):
                    inter = h2v[kb][:, s, 1:33, 1:33]
                    nc.vector.tensor_scalar(out=inter, in0=inter, scalar1=sc2[:, kb:kb + 1],
                                            scalar2=sh2[:, kb:kb + 1], op0=MULT, op1=ADD)
                    nc.vector.scalar_tensor_tensor(out=inter, in0=inter, scalar=0.2,
                                                   in1=inter, op0=MULT, op1=MAX)
            ph3 = tc.alloc_tile_pool(name="ph3", bufs=1, side="right")
            h3 = [ph3.tile([128, spc, 361], F32R, tag=f"h3_{kb}", name=f"h3_{kb}")
                  for kb in range(4)]
            h3v = [h.rearrange("p s (h w) -> p s h w", h=19) for h in h3]
            for kb in range(4):
                zf(h3v[kb][:, :, 0, :])
                zf(h3v[kb][:, :, 17, :])
                zf(h3v[kb][:, :, 18, :])
                zf(h3v[kb][:, :, 1:17, 0])
                zf(h3v[kb][:, :, 1:17, 17])
                zf(h3v[kb][:, :, 1:17, 18])
            with tc.tile_pool(name="pc_w", bufs=4) as pc_w, \
                 tc.tile_pool(name="pc_scr", bufs=2) as pc_scr, \
                 tc.tile_pool(name="ps_l3", bufs=8, space="PSUM") as ps_l3:
                s3p, s3q = part[3]
                for mt in range(4):
                    w3t = [pc_w.tile([128, 16, 128], F32R, tag="w3", name="w3t") for _ in range(2)]
                    for kb in range(2):
                        nc.sync.dma_start(out=w3t[kb][:], in_=w3_d[mt, kb])
                    pss = [ps_l3.tile([128, 512], F32, tag="l3", name="psl3")
                           for _ in range(spc // 2)]
                    for kb in range(2):
                        for tap, (ky, kx) in enumerate(_TAPS):
                            lhsT = w3t[kb][:, tap, :]
                            for s2 in range(spc // 2):
                                rhs = h2v[kb][:, 2 * s2:2 * s2 + 2, ky:ky + 31:2, kx:kx + 31:2]
                                out = pss[s2][:].rearrange("p (s a b) -> p s a b", s=2, a=16)
                                nc.tensor.matmul(out, lhsT, rhs,
                                                 start=(kb == 0 and tap == 0),
                                                 stop=(kb == 1 and tap == 15))
                    for s2 in range(spc // 2):
                        psv = pss[s2][:].rearrange("p (s n) -> p s n", s=2)
                        for sh_ in range(2):
                            s = 2 * s2 + sh_
                            dst = h3v[mt][:, s, 1:17, 1:17]
                            nc.scalar.activation(out=dst,
                                                 in_=psv[:, sh_, :].rearrange("p (a b) -> p a b", a=16),
                                                 func=ACTF.Copy, accum_out=s3p[:, mt, s:s + 1])
                            scr = pc_scr.tile([128, 512], F32, tag="scr", name="scr")
                            nc.scalar.activation(out=scr[:, 0:256], in_=psv[:, sh_, :], func=ACTF.Square,
                                                 accum_out=s3q[:, mt, s:s + 1])
            ph2.release()

            bn_reduce_allreduce(3, 4, N3)

            # ============================ PHASE D ============================
            sc3, sh3 = scsh[3]
            for kb in range(4):
                for s in range(spc):
                    inter = h3v[kb][:, s, 1:17, 1:17]
                    nc.vector.tensor_scalar(out=inter, in0=inter, scalar1=sc3[:, kb:kb + 1],
                                            scalar2=sh3[:, kb:kb + 1], op0=MULT, op1=ADD)
                    nc.vector.scalar_tensor_tensor(out=inter, in0=inter, scalar=0.2,
                                                   in1=inter, op0=MULT, op1=MAX)
            ph4 = tc.alloc_tile_pool(name="ph4", bufs=1)
            h4 = [ph4.tile([128, spc, 361], F32R, tag=f"h4_{kb}", name=f"h4_{kb}")
                  for kb in range(4)]
            h4v = [h.rearrange("p s (h w) -> p s h w", h=19) for h in h4]
            for kb in range(4):
                zf(h4v[kb][:, :, 0, :])
                zf(h4v[kb][:, :, 17, :])
                zf(h4v[kb][:, :, 18, :])
                zf(h4v[kb][:, :, 1:17, 0])
                zf(h4v[kb][:, :, 1:17, 17])
                zf(h4v[kb][:, :, 1:17, 18])
            with tc.tile_pool(name="pd_w", bufs=6) as pd_w, \
                 tc.tile_pool(name="pd_scr", bufs=2) as pd_scr, \
                 tc.tile_pool(name="ps_l4", bufs=8, space="PSUM") as ps_l4:
                s4p, s4q = part[4]
                for mt in range(4):
                    w4t = [pd_w.tile([128, 16, 128], F32R, tag="w4", name="w4t") for _ in range(4)]
                    for kb in range(4):
                        nc.sync.dma_start(out=w4t[kb][:], in_=w4_d[mt, kb])
                    pss = [ps_l4.tile([128, 512], F32, tag="l4", name="psl4")
                           for _ in range(spc // 2)]
                    for kb in range(4):
                        for tap, (ky, kx) in enumerate(_TAPS):
                            lhsT = w4t[kb][:, tap, :]
                            for s2 in range(spc // 2):
                                rhs = h3v[kb][:, 2 * s2:2 * s2 + 2, ky:ky + 16, kx:kx + 16]
                                out = pss[s2][:].rearrange("p (s a b) -> p s a b", s=2, a=16)
                                nc.tensor.matmul(out, lhsT, rhs,
                                                 start=(kb == 0 and tap == 0),
                                                 stop=(kb == 3 and tap == 15))
                    for s2 in range(spc // 2):
                        psv = pss[s2][:].rearrange("p (s n) -> p s n", s=2)
                        for sh_ in range(2):
                            s = 2 * s2 + sh_
                            dst = h4v[mt][:, s, 1:17, 1:17]
                            nc.scalar.activation(out=dst,
                                                 in_=psv[:, sh_, :].rearrange("p (a b) -> p a b", a=16),
                                                 func=ACTF.Copy, accum_out=s4p[:, mt, s:s + 1])
                            scr = pd_scr.tile([128, 512], F32, tag="scr", name="scr")
                            nc.scalar.activation(out=scr[:, 0:256], in_=psv[:, sh_, :], func=ACTF.Square,
                                                 accum_out=s4q[:, mt, s:s + 1])
            ph3.release()

            bn_reduce_allreduce(4, 4, N4)

            # ============================ PHASE E ============================
            sc4, sh4 = scsh[4]
            for kb in range(4):
                for s in range(spc):
                    inter = h4v[kb][:, s, 1:17, 1:17]
                    nc.vector.tensor_scalar(out=inter, in0=inter, scalar1=sc4[:, kb:kb + 1],
                                            scalar2=sh4[:, kb:kb + 1], op0=MULT, op1=ADD)
                    nc.vector.scalar_tensor_tensor(out=inter, in0=inter, scalar=0.2,
                                                   in1=inter, op0=MULT, op1=MAX)
            with tc.tile_pool(name="pe_out", bufs=1) as pe_out, \
                 tc.tile_pool(name="ps_l5", bufs=8, space="PSUM") as ps_l5:
                ys = pe_out.tile([1, spc, 256], F32)
                pse = [ps_l5.tile([1, 512], F32, tag="l5", name="psl5")
                       for s in range(spc // 2)]
                for kb in range(4):
                    for tap, (ky, kx) in enumerate(_TAPS):
                        lhsT = w5_sb[:, kb, tap:tap + 1]
                        for s2 in range(spc // 2):
                            rhs = h4v[kb][:, 2 * s2:2 * s2 + 2, ky:ky + 16, kx:kx + 16]
                            out = pse[s2][:].rearrange("p (s a b) -> p s a b", s=2, a=16)
                            nc.tensor.matmul(out, lhsT, rhs,
                                             start=(kb == 0 and tap == 0),
                                             stop=(kb == 3 and tap == 15))
                for s2 in range(spc // 2):
                    psv = pse[s2][:].rearrange("p (s n) -> p s n", s=2)
                    for sh_ in range(2):
                        nc.scalar.activation(out=ys[:, 2 * s2 + sh_, :], in_=psv[:, sh_, :],
                                             func=ACTF.Copy)
                nc.sync.dma_start(out=y_d[:], in_=ys[:])
            ph4.release()
    nc.compile()
    return nc


# --------------------------------------------------------------------------
# Host-side input prep
# --------------------------------------------------------------------------
def _prep_inputs(x, params, spc=8, cores=N_CORES):
    x = np.ascontiguousarray(np.asarray(x, np.float32))
    p = {k: np.asarray(v, np.float32) for k, v in params.items()}

    ends = np.stack([_rasterize_host(x[i, 1]) for i in range(x.shape[0])])
    xin = np.zeros((x.shape[0], 2, 258, 258), np.float32)
    xin[:, 0, 1:257, 1:257] = x[:, 0]
    xin[:, 1, 1:257, 1:257] = ends
    # im2col rows ordered (ky, kx, ci)
    cols = np.empty((x.shape[0], 4, 4, 2, 128, 128), np.float32)
    for ky in range(4):
        for kx in range(4):
            cols[:, ky, kx] = xin[:, :, ky:ky + 255:2, kx:kx + 255:2]
    imcol = cols.reshape(x.shape[0], 32, 16384)

    w0 = np.tile(p["w0"].transpose(2, 3, 1, 0).reshape(32, 64), (4, 1))
    w1 = np.tile(p["w1"].transpose(1, 2, 3, 0).reshape(64, 16, 128), (2, 1, 1))
    w2 = p["w2"].transpose(1, 2, 3, 0).reshape(128, 16, 256)
    w3 = (p["w3"].transpose(1, 2, 3, 0).reshape(2, 128, 16, 4, 128)
          .transpose(3, 0, 1, 2, 4))
    w4 = (p["w4"].transpose(1, 2, 3, 0).reshape(4, 128, 16, 4, 128)
          .transpose(3, 0, 1, 2, 4))
    w5 = p["w5"].transpose(1, 2, 3, 0).reshape(4, 128, 16).transpose(1, 0, 2)

    def bn_pack(li, nb):
        g = p[f"g{li}"].reshape(nb, 128)
        be = p[f"be{li}"].reshape(nb, 128)
        t = np.empty((128, nb, 2), np.float32)
        t[:, :, 0] = g.T
        t[:, :, 1] = be.T
        return t

    # NOTE: conv biases b1..b4 cancel inside training-mode BN (they shift the
    # mean by exactly b); b5 is added on the host after the gather; b0 is zero
    # in this problem's setup (asserted in kernel()).
    shared = {
        "w0": np.ascontiguousarray(w0), "w1": np.ascontiguousarray(w1),
        "w2": np.ascontiguousarray(w2), "w3": np.ascontiguousarray(w3),
        "w4": np.ascontiguousarray(w4), "w5": np.ascontiguousarray(w5),
        "bn1": bn_pack(1, 1), "bn2": bn_pack(2, 2),
        "bn3": bn_pack(3, 4), "bn4": bn_pack(4, 4),
    }
    in_maps = []
    for c in range(cores):
        sm = imcol[c * spc:(c + 1) * spc]  # [spc, 32, 16384]
        x0 = np.empty((spc // 2, 128, 8192), np.float32)
        for pr in range(spc // 2):
            x0[pr, 0:32] = sm[2 * pr, :, :8192]
            x0[pr, 32:64] = sm[2 * pr + 1, :, :8192]
            x0[pr, 64:96] = sm[2 * pr, :, 8192:]
            x0[pr, 96:128] = sm[2 * pr + 1, :, 8192:]
        m = dict(shared)
        m["x0"] = np.ascontiguousarray(x0)
        in_maps.append(m)
    return in_maps, p


_NC_CACHE = {}


def kernel(x, params):
    global LAST_EXEC_NS
    in_maps, p = _prep_inputs(x, params, spc=SPC)
    assert not np.any(p["b0"]), "conv0 bias path not implemented"
    key = (SPC,)
    if key not in _NC_CACHE:
        _NC_CACHE[key] = build_nc(spc=SPC)
    nc = _NC_CACHE[key]
    res = run_bass_kernel_spmd(
        nc, in_maps, list(range(N_CORES)),
        trace=TRACE, tmpdir=TRACE_TMPDIR,
    )
    LAST_EXEC_NS = res.exec_time_ns
    out = np.concatenate([res.results[c]["y"].reshape(SPC, 256) for c in range(N_CORES)], axis=0)
    out = out.reshape(B, 1, 16, 16) + p["b5"].reshape(1, 1, 1, 1)
    return out.astype(np.float32)
